# revision 1
# baseline (speedup 1.0000x reference)
"""JointAngleLoss Trainium2 kernel (8-core data-parallel).

Input : pose23d_pred [524288, 21, 3] float32
Output: scalar float32 loss (matches reference.reference)

Strategy: pure data-parallel over the batch dim; each of 8 NeuronCores handles
65536 rows. Host pre-permutes the input (dtype preserving) into a per-partition
slot layout J[c][jj][f][k] (jj = joint-within-finger, duplicating the 4 shared
joints: 75 floats per row) so that EVERY device-side vector operand is a flat
contiguous fp16 slice - this keeps the DVE in its 2x_1P packed perf mode.
Device pipeline per group:
  DMA fp32 -> ACT cast->fp16 -> DVE bones/crosses/dots (flat fp16 2x)
  -> ACT relu(-v)^2 with fp32 accum_out  +  PE ones-matmul reduces the
  coplanarity products into PSUM fp32 across groups.
Host sums the per-core partials in float64.
"""

import sys

for _p in ("/opt/trn_rl_repo", "/root/.axon_site/_ro/trn_rl_repo"):
    if _p not in sys.path:
        sys.path.append(_p)

import numpy as np

import concourse.bacc as bacc
import concourse.mybir as mybir
from concourse import tile
from concourse.bass_utils import run_bass_kernel_spmd
from contextlib import ExitStack

N_CORES = 8
P = 128          # SBUF partitions
B_FULL = 524288  # total batch
ROW = 75         # 3 comps * 5 joint-slots * 5 fingers (shared joints duplicated)

F16 = mybir.dt.float16
F32 = mybir.dt.float32


def build_bass(rows_per_core: int, K: int, reps: int = 1, hw_loop: int = 1,
               pool_bones: bool = False):
    """rows_per_core = P * K * G.  K = rows per partition slot per group.

    reps>1 unrolls the compute (timing); hw_loop>1 wraps it in a device-side
    For_i (timing; outputs = last iteration's = one correct pass).
    """
    assert rows_per_core % (P * K) == 0
    G = rows_per_core // (P * K)
    FK = ROW * K          # fp32 elems per partition per group (75*K)
    CJ = 25 * K           # joint elems per component (5jj*5f*K)
    CB = 20 * K           # bone elems per component  (4jj*5f*K)
    S5 = 5 * K            # one [f][k] slab
    NCOP = 3 * S5         # coplane products per partition
    NV = 2 * S5           # v values per partition

    nc = bacc.Bacc("TRN2", target_bir_lowering=False, debug=False)

    x = nc.dram_tensor("x", [G, P, FK], F32, kind="ExternalInput")
    cop_out = nc.dram_tensor("cop_out", [1, NCOP], F32, kind="ExternalOutput")
    mask_out = nc.dram_tensor("mask_out", [P, G * reps], F32, kind="ExternalOutput")

    with tile.TileContext(nc) as tc, ExitStack() as ctx:
        xpool = ctx.enter_context(tc.tile_pool(name="xpool", bufs=2))
        hpool = ctx.enter_context(tc.tile_pool(name="hpool", bufs=1))
        bpool = ctx.enter_context(tc.tile_pool(name="bpool", bufs=2))
        rpool = ctx.enter_context(tc.tile_pool(name="rpool", bufs=2))
        tpool = ctx.enter_context(tc.tile_pool(name="tpool", bufs=1))
        vpool = ctx.enter_context(tc.tile_pool(name="vpool", bufs=2))
        spool = ctx.enter_context(tc.tile_pool(name="spool", bufs=1))
        psum = ctx.enter_context(tc.tile_pool(name="psum", bufs=1, space="PSUM"))

        ones = spool.tile([P, 1], F16)
        nc.gpsimd.memset(ones[:], 1.0)
        acc = spool.tile([P, G * reps], F32)
        psum_cop = psum.tile([1, NCOP], F32)

        n_chunks = (NCOP + 511) // 512

        loop_cm = tc.For_i(0, hw_loop, 1) if hw_loop > 1 else None
        if loop_cm is not None:
            loop_cm.__enter__()

        for rep in range(reps):
            for g in range(G):
                first = rep == 0 and g == 0
                last = rep == reps - 1 and g == G - 1

                # ---- load + cast (all flat, split in half for earlier start)
                xt = xpool.tile([P, FK], F32)
                xh = hpool.tile([P, FK], F16)
                half = FK // 2
                for h in range(2):
                    sl = slice(h * half, (h + 1) * half)
                    nc.sync.dma_start(xt[:, sl], x.ap()[g][:, sl])
                    nc.scalar.copy(xh[:, sl], xt[:, sl])

                # ---- bones: B[c][jj][f][k] = J[c][jj+1][f][k]-J[c][jj][f][k]
                bones = bpool.tile([P, 3 * CB], F16)
                beng = nc.gpsimd if pool_bones else nc.vector
                for c in range(3):
                    beng.tensor_sub(
                        bones[:, c * CB : (c + 1) * CB],
                        xh[:, c * CJ + S5 : c * CJ + CJ],
                        xh[:, c * CJ : c * CJ + CB],
                    )

                def bslab(c, jj):  # bone block, flat [P, 5K]
                    o = c * CB + jj * S5
                    return bones[:, o : o + S5]

                # ---- crosses: R_c[qh][f][k], qh: 0=palm 1=mid 2=tip ---------
                # rot[c] = B_{c1}[jj=qh+1]*B_{c2}[jj=qh] - B_{c2}[jj=qh+1]*B_{c1}[jj=qh]
                rot = []
                for c in range(3):
                    c1, c2 = (c + 1) % 3, (c + 2) % 3
                    m1 = tpool.tile([P, NCOP], F16, tag="m1")
                    m2 = tpool.tile([P, NCOP], F16, tag="m2")
                    rc = rpool.tile([P, NCOP], F16, tag=f"rot{c}")
                    nc.vector.tensor_mul(
                        m1[:], bones[:, c1 * CB + S5 : c1 * CB + CB],
                        bones[:, c2 * CB : c2 * CB + NCOP])
                    nc.vector.tensor_mul(
                        m2[:], bones[:, c2 * CB + S5 : c2 * CB + CB],
                        bones[:, c1 * CB : c1 * CB + NCOP])
                    nc.vector.tensor_sub(rc[:], m1[:], m2[:])
                    rot.append(rc)

                # ---- coplane products: (palm_c + mid_c) * b4_c  (all flat) --
                red = vpool.tile([P, NCOP], F16, tag="red")
                for c in range(3):
                    pc = tpool.tile([P, S5], F16, tag="pc")
                    nc.vector.tensor_add(pc[:], rot[c][:, 0:S5], rot[c][:, S5:2 * S5])
                    nc.vector.tensor_mul(
                        red[:, c * S5 : (c + 1) * S5], pc[:], bslab(c, 3))

                # ---- v1 = tip.mid, v2 = palm.mid ----------------------------
                pprod = []
                for c in range(3):
                    pp = tpool.tile([P, NV], F16, tag=f"pp{c}")
                    nc.vector.tensor_mul(
                        pp[:, 0:S5], rot[c][:, 2 * S5 : 3 * S5], rot[c][:, S5 : 2 * S5])
                    nc.vector.tensor_mul(
                        pp[:, S5:NV], rot[c][:, 0:S5], rot[c][:, S5 : 2 * S5])
                    pprod.append(pp)
                vsum = tpool.tile([P, NV], F16, tag="vsum")
                nc.vector.tensor_add(vsum[:], pprod[0][:], pprod[1][:])
                v = vpool.tile([P, NV], F16, tag="v")
                nc.vector.tensor_add(v[:], vsum[:], pprod[2][:])

                # ---- masked squares on ACT: sum(relu(-v)^2) -> acc ----------
                mrelu = vpool.tile([P, NV], F16, tag="mrelu")
                nc.scalar.activation(mrelu[:], v[:], mybir.ActivationFunctionType.Relu,
                                     scale=-1.0)
                sqj = vpool.tile([P, NV], F16, tag="sqj")
                nc.scalar.activation(sqj[:], mrelu[:],
                                     mybir.ActivationFunctionType.Square,
                                     accum_out=acc[:, rep * G + g : rep * G + g + 1])

                # ---- PE reduction of coplane products over partitions -------
                for i in range(n_chunks):
                    lo = 512 * i
                    hi = min(NCOP, lo + 512)
                    nc.tensor.matmul(psum_cop[:, lo:hi], ones[:], red[:, lo:hi],
                                     start=first, stop=last)

        if loop_cm is not None:
            loop_cm.__exit__(None, None, None)

        # ---- epilogue: PSUM -> SBUF -> DRAM ---------------------------------
        cop_sb = spool.tile([1, NCOP], F32)
        nc.scalar.copy(cop_sb[:], psum_cop[:])
        nc.sync.dma_start(cop_out.ap(), cop_sb[:])
        nc.sync.dma_start(mask_out.ap(), acc[:])

    nc.compile()
    return nc, G


def host_planarize(x: np.ndarray, n_cores: int, K: int) -> np.ndarray:
    """[B,21,3] f32 -> [cores, G, P, 75K] f32: slot layout [c][jj:5][f:5][k]."""
    B = x.shape[0]
    R = B // n_cores
    G = R // (P * K)
    xr = x.reshape(n_cores, G, P, K, 21, 3)
    jidx = (np.arange(5) * 4)[:, None] + np.arange(5)[None, :]  # [f, jj]
    xj = xr[:, :, :, :, jidx, :]                 # [cores,G,P,K,f,jj,3]
    xp = xj.transpose(0, 1, 2, 6, 5, 4, 3)       # [cores,G,P,c,jj,f,K]
    return np.ascontiguousarray(xp).reshape(n_cores, G, P, ROW * K)


_CACHE = {}


def _get_nc(rows_per_core: int, K: int):
    key = (rows_per_core, K)
    if key not in _CACHE:
        _CACHE[key] = build_bass(rows_per_core, K)
    return _CACHE[key]


def kernel(pose23d_pred: np.ndarray) -> np.ndarray:
    x = np.asarray(pose23d_pred, dtype=np.float32)
    assert x.shape == (B_FULL, 21, 3), x.shape
    K = 128
    R = B_FULL // N_CORES
    nc, G = _get_nc(R, K)
    xp = host_planarize(x, N_CORES, K)
    in_maps = [{"x": xp[i]} for i in range(N_CORES)]
    res = run_bass_kernel_spmd(nc, in_maps, list(range(N_CORES)))
    total = 0.0
    for r in res.results:
        total += r["cop_out"].astype(np.float64).sum()
        total += r["mask_out"].astype(np.float64).sum()
    return np.float32(total)



# revision 3
# speedup vs baseline: 1.0613x; 1.0613x over previous
"""JointAngleLoss Trainium2 kernel (8-core data-parallel), v2.

Input : pose23d_pred [524288, 21, 3] float32
Output: scalar float32 loss (matches reference.reference)

Strategy: pure data-parallel over the batch dim; each of 8 NeuronCores handles
65536 rows. Host pre-permutes the input into a per-partition slot layout
J[c][jj][f][k] (jj = joint-within-finger, duplicating the 4 shared joints:
75 values per row) and casts to fp16 (loss tolerance 2e-2; measured fp16
input-cast error ~2e-6 relative), so every device-side vector operand is a
contiguous fp16 slice (DVE 2x_1P packed mode) and DMA bytes are halved.

Device pipeline per group:
  DMA fp16 -> DVE bones/crosses/dots (16 instrs, multi-dim APs)
  -> ACT relu(-v), square with fp32 accum_out
  -> PE ones-matmul reduces the coplanarity products into PSUM fp32.
Host sums the per-core partials in float64.
"""

import sys

for _p in ("/opt/trn_rl_repo", "/root/.axon_site/_ro/trn_rl_repo"):
    if _p not in sys.path:
        sys.path.append(_p)

import numpy as np

import concourse.bacc as bacc
import concourse.mybir as mybir
from concourse import tile
from concourse.bass_utils import run_bass_kernel_spmd
from contextlib import ExitStack

N_CORES = 8
P = 128          # SBUF partitions
B_FULL = 524288  # total batch
ROW = 75         # 3 comps * 5 joint-slots * 5 fingers (shared joints duplicated)

F16 = mybir.dt.float16
F32 = mybir.dt.float32


def build_bass(rows_per_core: int, K: int, reps: int = 1, hw_loop: int = 1,
               pool_units: int = 0):
    """rows_per_core = P * K * G.  K = rows per partition slot per group.

    reps>1 unrolls the compute (timing); hw_loop>1 wraps it in a device-side
    For_i (timing; outputs = last iteration's = one correct pass).
    pool_units in {0,4,7,10} moves part of the elementwise work to GpSimd.
    """
    assert rows_per_core % (P * K) == 0
    G = rows_per_core // (P * K)
    FK = ROW * K          # fp16 elems per partition per group (75*K)
    CJ = 25 * K           # joint elems per component (5jj*5f*K)
    CB = 20 * K           # bone elems per component  (4jj*5f*K)
    S5 = 5 * K            # one [f][k] slab
    NR = 9 * S5           # 3c * 3q * S5: m1/m2/rot elems per partition
    NCOP = 3 * S5         # coplane products per partition
    NV = 2 * S5           # v values per partition

    nc = bacc.Bacc("TRN2", target_bir_lowering=False, debug=False)

    x = nc.dram_tensor("x", [G, P, FK], F16, kind="ExternalInput")
    cop_out = nc.dram_tensor("cop_out", [1, NCOP], F32, kind="ExternalOutput")
    mask_out = nc.dram_tensor("mask_out", [P, G * reps], F32, kind="ExternalOutput")

    with tile.TileContext(nc) as tc, ExitStack() as ctx:
        xpool = ctx.enter_context(tc.tile_pool(name="xpool", bufs=2))
        bpool = ctx.enter_context(tc.tile_pool(name="bpool", bufs=2))
        mpool = ctx.enter_context(tc.tile_pool(name="mpool", bufs=1))
        rpool = ctx.enter_context(tc.tile_pool(name="rpool", bufs=2))
        vpool = ctx.enter_context(tc.tile_pool(name="vpool", bufs=2))
        spool = ctx.enter_context(tc.tile_pool(name="spool", bufs=1))
        psum = ctx.enter_context(tc.tile_pool(name="psum", bufs=1, space="PSUM"))

        ones = spool.tile([P, 1], F16)
        nc.gpsimd.memset(ones[:], 1.0)
        acc = spool.tile([P, G * reps], F32)
        psum_cop = psum.tile([1, NCOP], F32)

        n_chunks = (NCOP + 511) // 512

        def cview(t, n):  # [P, 3, n-block] view of a c-major tile
            return t[:].rearrange("p (c n) -> p c n", c=3)

        loop_cm = tc.For_i(0, hw_loop, 1) if hw_loop > 1 else None
        if loop_cm is not None:
            loop_cm.__enter__()

        for rep in range(reps):
            for g in range(G):
                first = rep == 0 and g == 0
                last = rep == reps - 1 and g == G - 1

                # ---- load fp16 (split in half for earlier compute start)
                xh = xpool.tile([P, FK], F16)
                half = FK // 2
                for h in range(2):
                    sl = slice(h * half, (h + 1) * half)
                    nc.sync.dma_start(xh[:, sl], x.ap()[g][:, sl])

                # ---- bones: B[c][jj][f][k] = J[c][jj+1][f][k]-J[c][jj][f][k]
                bones = bpool.tile([P, 3 * CB], F16)
                for c in range(3):
                    eng = nc.gpsimd if (pool_units >= 4 and c == 2) else nc.vector
                    eng.tensor_sub(
                        bones[:, c * CB : (c + 1) * CB],
                        xh[:, c * CJ + S5 : c * CJ + CJ],
                        xh[:, c * CJ : c * CJ + CB],
                    )

                # ---- cross products, c-major [c][q][f][k] -------------------
                # rot[c][q] = B_{c1}[q+1]*B_{c2}[q] - B_{c2}[q+1]*B_{c1}[q]
                m1 = mpool.tile([P, NR], F16, tag="m1")
                m2 = mpool.tile([P, NR], F16, tag="m2")
                rot = rpool.tile([P, NR], F16, tag="rot")
                for c in range(3):
                    c1, c2 = (c + 1) % 3, (c + 2) % 3
                    e1 = nc.gpsimd if (pool_units >= 10 and c == 2) else nc.vector
                    e2 = nc.gpsimd if (pool_units >= 7 and c == 2) else nc.vector
                    e1.tensor_mul(
                        m1[:, c * NCOP : (c + 1) * NCOP],
                        bones[:, c1 * CB + S5 : c1 * CB + CB],
                        bones[:, c2 * CB : c2 * CB + NCOP])
                    e2.tensor_mul(
                        m2[:, c * NCOP : (c + 1) * NCOP],
                        bones[:, c2 * CB + S5 : c2 * CB + CB],
                        bones[:, c1 * CB : c1 * CB + NCOP])
                nc.vector.tensor_sub(rot[:], m1[:], m2[:])

                rv = cview(rot, NR)       # [P, 3, 3*S5] (q within c)
                bv = cview(bones, 3 * CB)  # [P, 3, CB]  (jj within c)

                def qb(q):  # q-th cross block for each c: [P, 3, S5]
                    return rv[:, :, q * S5 : (q + 1) * S5]

                # ---- coplane products: (palm + mid)_c * b4_c ---------------
                pc = vpool.tile([P, NCOP], F16, tag="pc")
                red = vpool.tile([P, NCOP], F16, tag="red")
                nc.vector.tensor_add(cview(pc, NCOP), qb(0), qb(1))
                nc.vector.tensor_mul(
                    cview(red, NCOP), cview(pc, NCOP),
                    bv[:, :, 3 * S5 : 4 * S5])

                # ---- v1 = tip.mid, v2 = palm.mid; pp[c] = [v1_c | v2_c] ----
                pp = vpool.tile([P, 6 * S5], F16, tag="pp")
                ppv = cview(pp, 6 * S5)   # [P, 3, 2*S5]
                nc.vector.tensor_mul(ppv[:, :, 0:S5], qb(2), qb(1))
                nc.vector.tensor_mul(ppv[:, :, S5 : 2 * S5], qb(0), qb(1))
                vs = vpool.tile([P, NV], F16, tag="vs")
                v = vpool.tile([P, NV], F16, tag="v")
                nc.vector.tensor_add(vs[:], pp[:, 0:NV], pp[:, NV : 2 * NV])
                nc.vector.tensor_add(v[:], vs[:], pp[:, 2 * NV : 3 * NV])

                # ---- masked squares on ACT: sum(relu(-v)^2) -> acc ----------
                mrelu = vpool.tile([P, NV], F16, tag="mrelu")
                nc.scalar.activation(mrelu[:], v[:], mybir.ActivationFunctionType.Relu,
                                     scale=-1.0)
                sqj = vpool.tile([P, NV], F16, tag="sqj")
                nc.scalar.activation(sqj[:], mrelu[:],
                                     mybir.ActivationFunctionType.Square,
                                     accum_out=acc[:, rep * G + g : rep * G + g + 1])

                # ---- PE reduction of coplane products over partitions -------
                for i in range(n_chunks):
                    lo = 512 * i
                    hi = min(NCOP, lo + 512)
                    nc.tensor.matmul(psum_cop[:, lo:hi], ones[:], red[:, lo:hi],
                                     start=first, stop=last)

        if loop_cm is not None:
            loop_cm.__exit__(None, None, None)

        # ---- epilogue: PSUM -> SBUF -> DRAM ---------------------------------
        cop_sb = spool.tile([1, NCOP], F32)
        nc.scalar.copy(cop_sb[:], psum_cop[:])
        nc.sync.dma_start(cop_out.ap(), cop_sb[:])
        nc.sync.dma_start(mask_out.ap(), acc[:])

    nc.compile()
    return nc, G


def host_planarize(x: np.ndarray, n_cores: int, K: int) -> np.ndarray:
    """[B,21,3] f32 -> [cores, G, P, 75K] f16: slot layout [c][jj:5][f:5][k]."""
    B = x.shape[0]
    R = B // n_cores
    G = R // (P * K)
    xr = x.reshape(n_cores, G, P, K, 21, 3)
    jidx = (np.arange(5) * 4)[:, None] + np.arange(5)[None, :]  # [f, jj]
    xj = xr[:, :, :, :, jidx, :]                 # [cores,G,P,K,f,jj,3]
    xp = xj.transpose(0, 1, 2, 6, 5, 4, 3)       # [cores,G,P,c,jj,f,K]
    out = np.empty((n_cores, G, P, ROW * K), dtype=np.float16)
    np.copyto(out.reshape(xp.shape), xp)
    return out


_CACHE = {}


def _get_nc(rows_per_core: int, K: int):
    key = (rows_per_core, K)
    if key not in _CACHE:
        _CACHE[key] = build_bass(rows_per_core, K)
    return _CACHE[key]


def kernel(pose23d_pred: np.ndarray) -> np.ndarray:
    x = np.asarray(pose23d_pred, dtype=np.float32)
    assert x.shape == (B_FULL, 21, 3), x.shape
    K = 128
    R = B_FULL // N_CORES
    nc, G = _get_nc(R, K)
    xp = host_planarize(x, N_CORES, K)
    in_maps = [{"x": xp[i]} for i in range(N_CORES)]
    res = run_bass_kernel_spmd(nc, in_maps, list(range(N_CORES)))
    total = 0.0
    for r in res.results:
        total += r["cop_out"].astype(np.float64).sum()
        total += r["mask_out"].astype(np.float64).sum()
    return np.float32(total)
